# revision 10
# baseline (speedup 1.0000x reference)
"""Trainium2 Bass kernel: image -> additive-sinusoid audio encoding.

Math (per batch image b):
  gray = 255 * (w . rgb);  rev = flip(gray, rows);  avg = mean(gray)
  px   = clip(3*rev - 2*avg, 0, 255)
  A    = where(px==0, 0, exp(ln10 * (px/160 - 1.5)))            # [M=64 rows, N=64 cols]
  y[t] = sum_m A[m, col(t)] * sin(W[m]*t*dt + PHI0[m]),  col(t) = min(t//361, 63)
  audio= clip(0.5 + 2048*y, -32768, 32767)                       # [ns=23152]

Kernel strategy: t = n*361 + r  =>  angle = theta[i,n] + beta[i,r] (row flip folded
into the host tables), so  sinmat = sin(theta)cos(beta) + cos(theta)sin(beta) and
the gathered einsum becomes dense fp16 matmuls of P/Q = A*sin(theta)/A*cos(theta)
against tiny constant cos/sin(beta) banks. Data-parallel over batch: 8 images per
NeuronCore, layout [128 partitions = (batch-half, image-row), 256 = (b2, col)].
"""

import os

import numpy as np

# ---- problem constants (from the nn.Module definition; input-independent) ----
M = 64
N = 64
FL, FH, FS, T = 80.0, 7600.0, 22050, 1.05
NS = 2 * int(0.5 * FS * T)  # 23152
NUM = NS // N  # 361
RMAX = NS - (N - 1) * NUM  # 409 (last column's sample count)
DT = float(np.float32(1.0 / FS))  # reference rounds dt to f32 (jnp weak typing)
TWO_PI = 2.0 * np.pi
B = 64
N_CORES = 8
B_LOC = B // N_CORES  # 8 images per core
SCALE_SSM = (0.5 / np.sqrt(M)) * 32768.0  # 2048
LN10 = float(np.log(10.0))
EXP_A = LN10 / 160.0
EXP_B = -1.5 * LN10
W0, W1, W2 = 0.2989, 0.5870, 0.1140
C00 = 3.0 * 255.0 * W0  # fold of the 3*255*w0 scale into the gray accumulator
R1 = W1 / W0
R2 = W2 / W0
KAVG2 = 2.0 * 255.0 * W0 / 4096.0  # sum(t) -> 2*avg(gray255) weighting


def _make_tables():
    # LCG phase bank (faithful port, ir starts at 0)
    ia, ic, im = 9301, 49297, 233280
    ir = 0
    phi = []
    for _ in range(M):
        ir = (ir * ia + ic) % im
        phi.append(TWO_PI * ir / im)
    phi32 = np.array(phi, np.float64).astype(np.float32)
    w32 = (TWO_PI * FL * (FH / FL) ** (np.arange(M) / (M - 1))).astype(np.float32)

    # fold the row flip (tf.reverse on axis 1) into the tables: row i uses W[63-i]
    wf = w32[::-1].astype(np.float64)
    phif = phi32[::-1].astype(np.float64)

    n_idx = np.arange(N, dtype=np.float64)
    theta = wf[:, None] * (n_idx[None, :] * NUM * DT) + phif[:, None]  # [64, 64]
    st = np.sin(theta)
    ct = np.cos(theta)

    r_idx = np.arange(RMAX, dtype=np.float64)
    beta = wf[:, None] * (r_idx[None, :] * DT)  # [64, 409]
    cb = np.tile((SCALE_SSM * np.cos(beta)).astype(np.float16), (2, 1))  # [128, RMAX]
    sb = np.tile((SCALE_SSM * np.sin(beta)).astype(np.float16), (2, 1))

    # [p=(bh,i), (b2,n)] broadcast of the theta tables
    stbc = np.tile(st[None, :, None, :], (2, 1, 4, 1)).reshape(128, 256)
    ctbc = np.tile(ct[None, :, None, :], (2, 1, 4, 1)).reshape(128, 256)

    halfsel = np.zeros((128, 2), np.float32)
    halfsel[:64, 0] = 1.0
    halfsel[64:, 1] = 1.0

    # pack1: [halfsel(2) | stbc(256) | ctbc(256) | expb(1) | halfb(1) | halfones2(128)]
    pack1 = np.zeros((128, 644), np.float32)
    pack1[:, 0:2] = halfsel
    pack1[:, 2:258] = stbc
    pack1[:, 258:514] = ctbc
    pack1[:, 514] = EXP_B
    pack1[:, 515] = 0.5
    pack1[0, 516:580] = 1.0   # halfones2[0, p] = (p//64 == 0)
    pack1[1, 580:644] = 1.0   # halfones2[1, p] = (p//64 == 1)

    # pack2: [cb | sb] fp16
    pack2 = np.concatenate([cb, sb], axis=1)  # [128, 2*RMAX]

    return {"pack1": pack1, "pack2": pack2}


_TABLES = None


def tables():
    global _TABLES
    if _TABLES is None:
        _TABLES = _make_tables()
    return _TABLES


def build_nc():
    import concourse.bacc as bacc
    import concourse.bass as bass
    import concourse.mybir as mybir
    import concourse.tile as tile

    f32 = mybir.dt.float32
    f16 = mybir.dt.float16
    Alu = mybir.AluOpType
    Act = mybir.ActivationFunctionType

    nc = bacc.Bacc(
        "TRN2",
        target_bir_lowering=False,
        debug=False,
        num_devices=N_CORES,
        enable_asserts=False,
    )

    x_d = nc.dram_tensor("x", [B_LOC, 64, 64, 3], f32, kind="ExternalInput")
    pack1_d = nc.dram_tensor("pack1", [128, 644], f32, kind="ExternalInput")
    pack2_d = nc.dram_tensor("pack2", [128, 2 * RMAX], f16, kind="ExternalInput")
    audio_d = nc.dram_tensor("audio", [B_LOC, NS], f32, kind="ExternalOutput")

    with tile.TileContext(nc) as tc:
        with (
            tc.tile_pool(name="consts", bufs=1) as consts,
            tc.tile_pool(name="work", bufs=1) as work,
            tc.tile_pool(name="outp", bufs=4) as outp,
            tc.tile_pool(name="psum_y", bufs=4, space=bass.MemorySpace.PSUM) as psum_y,
            tc.tile_pool(name="psum_m", bufs=1, space=bass.MemorySpace.PSUM) as psum_m,
        ):
            # ---- input image: [p=(bh,i), (b2, j, c)]; one DMA per batch half,
            # split across the two physical HWDGE rings (sync / scalar) ----
            X = work.tile([128, 768], f32)
            xv = x_d[:].rearrange("(bh b2) i j c -> bh i b2 j c", bh=2)
            Xv = X[:].rearrange("(bh i) (b2 j c) -> bh i b2 j c", bh=2, b2=4, c=3)
            nc.sync.dma_start(out=Xv[0], in_=xv[0])
            nc.scalar.dma_start(out=Xv[1], in_=xv[1])

            # ---- constant tables (packed; one DMA per ring) ----
            pk1 = consts.tile([128, 644], f32)
            pk2 = consts.tile([128, 2 * RMAX], f16)
            nc.sync.dma_start(out=pk1, in_=pack1_d[:])
            nc.scalar.dma_start(out=pk2, in_=pack2_d[:])
            halfsel = pk1[:, 0:2]
            stbc = pk1[:, 2:258]
            ctbc = pk1[:, 258:514]
            expb = pk1[:, 514:515]
            halfb = pk1[:, 515:516]
            halfones2 = pk1[0:2, 516:644]
            cb = pk2[:, 0:RMAX]
            sbt = pk2[:, RMAX : 2 * RMAX]


            # ---- grayscale accumulate: t = R + r1*G + r2*B; rs = per-b2 row sums ----
            Xc = X[:].rearrange("p (q c) -> p q c", c=3)
            t = work.tile([128, 4, 64], f32)
            rs = work.tile([128, 4], f32)
            nc.vector.scalar_tensor_tensor(
                out=t.rearrange("p a b -> p (a b)"),
                in0=Xc[:, :, 1], scalar=float(R1), in1=Xc[:, :, 0],
                op0=Alu.mult, op1=Alu.add,
            )
            Xb = X[:].rearrange("p (a b c) -> p a b c", a=4, c=3)
            for b2 in range(4):
                nc.vector.scalar_tensor_tensor(
                    out=t[:, b2], in0=Xb[:, b2, :, 2], scalar=float(R2),
                    in1=t[:, b2], op0=Alu.mult, op1=Alu.add,
                    accum_out=rs[:, b2 : b2 + 1],
                )

            # ---- mean: cross-partition sum via PE, broadcast back via PE ----
            csS = psum_m.tile([2, 4, 1], f32)
            nc.tensor.matmul(csS, halfsel, rs, start=True, stop=True)
            Rb = work.tile([2, 4, 64], f32)
            nc.vector.tensor_scalar_mul(
                out=Rb, in0=csS.broadcast_to([2, 4, 64]), scalar1=float(KAVG2)
            )
            avgbc = psum_m.tile([128, 256], f32)
            nc.tensor.matmul(
                avgbc, halfones2, Rb[:].rearrange("p a b -> p (a b)"),
                start=True, stop=True,
            )

            # ---- px = clip(3*gray - 2*avg, 0, 255) ----
            px = work.tile([128, 256], f32)
            nc.vector.scalar_tensor_tensor(
                out=px, in0=t.rearrange("p a b -> p (a b)"), scalar=float(C00),
                in1=avgbc, op0=Alu.mult, op1=Alu.subtract,
            )
            nc.vector.tensor_scalar(
                out=px, in0=px, scalar1=0.0, scalar2=255.0,
                op0=Alu.max, op1=Alu.min,
            )

            # ---- A = (px > 0) * exp(EXP_A*px + EXP_B) ----
            E = work.tile([128, 256], f32)
            nc.scalar.activation(
                out=E, in_=px, func=Act.Exp, bias=expb, scale=float(EXP_A)
            )
            A = work.tile([128, 256], f32)
            nc.vector.scalar_tensor_tensor(
                out=A, in0=px, scalar=0.0, in1=E, op0=Alu.is_gt, op1=Alu.mult
            )

            # ---- P = A*sin(theta), Q = A*cos(theta)  (fp16 for the PE) ----
            P = work.tile([128, 256], f16)
            Q = work.tile([128, 256], f16)
            nc.vector.tensor_mul(out=P, in0=A, in1=stbc)
            nc.vector.tensor_mul(out=Q, in0=A, in1=ctbc)

            # ---- per 2-batch group: Y = P^T@CB + Q^T@SB ; +0.5 ; clip ; DMA ----
            for g in range(4):
                bh, s = divmod(g, 2)
                prt = slice(64 * bh, 64 * (bh + 1))
                col = slice(128 * s, 128 * (s + 1))
                y_ps = psum_y.tile([128, RMAX], f32, tag="y")
                nc.tensor.matmul(y_ps, P[prt, col], cb[prt], start=True, stop=False)
                nc.tensor.matmul(y_ps, Q[prt, col], sbt[prt], start=False, stop=True)

                u = outp.tile([128, RMAX], f32, tag="u")
                nc.scalar.activation(
                    out=u, in_=y_ps, func=Act.Identity, bias=halfb, scale=1.0
                )
                nc.vector.tensor_scalar(
                    out=u, in0=u, scalar1=-32768.0, scalar2=32767.0,
                    op0=Alu.max, op1=Alu.min,
                )

                # main block: all 128 partitions x 361 in one DMA
                b0 = 4 * bh + 2 * s
                nc.sync.dma_start(
                    out=bass.AP(audio_d, b0 * NS, [[NS, 2], [NUM, 64], [1, NUM]]),
                    in_=u[:, 0:NUM],
                )
                # tails: partition {63, 127} columns [361, 409) -> audio[b, 23104:]
                for half in range(2):
                    nc.gpsimd.dma_start(
                        out=bass.AP(
                            audio_d,
                            (b0 + half) * NS + N * NUM,
                            [[1, 1], [1, RMAX - NUM]],
                        ),
                        in_=u[64 * half + 63 : 64 * half + 64, NUM:RMAX],
                    )

    nc.compile()
    return nc


_NC = None


def _get_nc():
    global _NC
    if _NC is None:
        _NC = build_nc()
    return _NC


LAST_RESULTS = None


def kernel(x: np.ndarray) -> np.ndarray:
    from concourse.bass_utils import run_bass_kernel_spmd

    x = np.ascontiguousarray(np.asarray(x, dtype=np.float32))
    assert x.shape == (B, 64, 64, 3), x.shape

    nc = _get_nc()
    tbl = tables()
    in_maps = []
    for c in range(N_CORES):
        m = {"x": np.ascontiguousarray(x[c * B_LOC : (c + 1) * B_LOC])}
        m.update(tbl)
        in_maps.append(m)

    trace = os.environ.get("BASS_KERNEL_TRACE", "0") == "1"
    res = run_bass_kernel_spmd(
        nc, in_maps, core_ids=list(range(N_CORES)), trace=trace
    )
    global LAST_RESULTS
    LAST_RESULTS = res
    return np.concatenate([r["audio"] for r in res.results], axis=0)


# revision 12
# speedup vs baseline: 1.3873x; 1.3873x over previous
"""Trainium2 Bass kernel: image -> additive-sinusoid audio encoding.

Math (per batch image b):
  gray = 255 * (w . rgb);  rev = flip(gray, rows);  avg = mean(gray)
  px   = clip(3*rev - 2*avg, 0, 255)
  A    = where(px==0, 0, exp(ln10 * (px/160 - 1.5)))            # [M=64 rows, N=64 cols]
  y[t] = sum_m A[m, col(t)] * sin(W[m]*t*dt + PHI0[m]),  col(t) = min(t//361, 63)
  audio= clip(0.5 + 2048*y, -32768, 32767)                       # [ns=23152]

Kernel strategy: t = n*361 + r  =>  angle = theta[i,n] + beta[i,r] (row flip folded
into the host tables), so  sinmat = sin(theta)cos(beta) + cos(theta)sin(beta) and
the gathered einsum becomes dense fp16 matmuls of P/Q = A*sin(theta)/A*cos(theta)
against tiny constant cos/sin(beta) banks. Data-parallel over batch: 8 images per
NeuronCore, layout [128 partitions = (batch-half, image-row), 256 = (b2, col)].
"""

import os

import numpy as np

# ---- problem constants (from the nn.Module definition; input-independent) ----
M = 64
N = 64
FL, FH, FS, T = 80.0, 7600.0, 22050, 1.05
NS = 2 * int(0.5 * FS * T)  # 23152
NUM = NS // N  # 361
RMAX = NS - (N - 1) * NUM  # 409 (last column's sample count)
DT = float(np.float32(1.0 / FS))  # reference rounds dt to f32 (jnp weak typing)
TWO_PI = 2.0 * np.pi
B = 64
N_CORES = 8
B_LOC = B // N_CORES  # 8 images per core
SCALE_SSM = (0.5 / np.sqrt(M)) * 32768.0  # 2048
LN10 = float(np.log(10.0))
EXP_A = LN10 / 160.0
EXP_B = -1.5 * LN10
W0, W1, W2 = 0.2989, 0.5870, 0.1140
C00 = 3.0 * 255.0 * W0  # fold of the 3*255*w0 scale into the gray accumulator
R1 = W1 / W0
R2 = W2 / W0
KAVG2 = 2.0 * 255.0 * W0 / 4096.0  # sum(t) -> 2*avg(gray255) weighting


def _make_tables():
    # LCG phase bank (faithful port, ir starts at 0)
    ia, ic, im = 9301, 49297, 233280
    ir = 0
    phi = []
    for _ in range(M):
        ir = (ir * ia + ic) % im
        phi.append(TWO_PI * ir / im)
    phi32 = np.array(phi, np.float64).astype(np.float32)
    w32 = (TWO_PI * FL * (FH / FL) ** (np.arange(M) / (M - 1))).astype(np.float32)

    # fold the row flip (tf.reverse on axis 1) into the tables: row i uses W[63-i]
    wf = w32[::-1].astype(np.float64)
    phif = phi32[::-1].astype(np.float64)

    n_idx = np.arange(N, dtype=np.float64)
    theta = wf[:, None] * (n_idx[None, :] * NUM * DT) + phif[:, None]  # [64, 64]
    st = np.sin(theta)
    ct = np.cos(theta)

    r_idx = np.arange(RMAX, dtype=np.float64)
    beta = wf[:, None] * (r_idx[None, :] * DT)  # [64, 409]
    cb = np.tile((SCALE_SSM * np.cos(beta)).astype(np.float16), (2, 1))  # [128, RMAX]
    sb = np.tile((SCALE_SSM * np.sin(beta)).astype(np.float16), (2, 1))

    # [p=(bh,i), (b2,n)] broadcast of the theta tables
    stbc = np.tile(st[None, :, None, :], (2, 1, 4, 1)).reshape(128, 256)
    ctbc = np.tile(ct[None, :, None, :], (2, 1, 4, 1)).reshape(128, 256)

    halfsel = np.zeros((128, 2), np.float32)
    halfsel[:64, 0] = 1.0
    halfsel[64:, 1] = 1.0

    # pack1: [halfsel(2) | stbc(256) | ctbc(256) | expb(1) | halfb(1) | halfones2(128)]
    pack1 = np.zeros((128, 644), np.float32)
    pack1[:, 0:2] = halfsel
    pack1[:, 2:258] = stbc
    pack1[:, 258:514] = ctbc
    pack1[:, 514] = EXP_B
    pack1[:, 515] = 0.5
    pack1[0, 516:580] = 1.0   # halfones2[0, p] = (p//64 == 0)
    pack1[1, 580:644] = 1.0   # halfones2[1, p] = (p//64 == 1)

    # pack2: [cb | sb] fp16
    pack2 = np.concatenate([cb, sb], axis=1)  # [128, 2*RMAX]

    return {"pack1": pack1, "pack2": pack2}


_TABLES = None


def tables():
    global _TABLES
    if _TABLES is None:
        _TABLES = _make_tables()
    return _TABLES


def build_nc():
    import concourse.bacc as bacc
    import concourse.bass as bass
    import concourse.mybir as mybir
    import concourse.tile as tile

    f32 = mybir.dt.float32
    f16 = mybir.dt.float16
    Alu = mybir.AluOpType
    Act = mybir.ActivationFunctionType

    nc = bacc.Bacc(
        "TRN2",
        target_bir_lowering=False,
        debug=False,
        num_devices=N_CORES,
        enable_asserts=False,
    )

    x_d = nc.dram_tensor("x", [B_LOC, 64, 64, 3], f32, kind="ExternalInput")
    pack1_d = nc.dram_tensor("pack1", [128, 644], f32, kind="ExternalInput")
    pack2_d = nc.dram_tensor("pack2", [128, 2 * RMAX], f16, kind="ExternalInput")
    audio_d = nc.dram_tensor("audio", [B_LOC, NS], f32, kind="ExternalOutput")

    with tile.TileContext(nc) as tc:
        with (
            tc.tile_pool(name="consts", bufs=1) as consts,
            tc.tile_pool(name="work", bufs=1) as work,
            tc.tile_pool(name="outp", bufs=4) as outp,
            tc.tile_pool(name="psum_y", bufs=4, space=bass.MemorySpace.PSUM) as psum_y,
            tc.tile_pool(name="psum_m", bufs=1, space=bass.MemorySpace.PSUM) as psum_m,
        ):
            # ---- input image: [p=(bh,i), (b2, j, c)]; one DMA per batch half,
            # split across the two physical HWDGE rings (sync / scalar) ----
            X = work.tile([128, 768], f32)
            xv = x_d[:].rearrange("(bh b2) i j c -> bh i b2 j c", bh=2)
            Xv = X[:].rearrange("(bh i) (b2 j c) -> bh i b2 j c", bh=2, b2=4, c=3)
            nc.sync.dma_start(out=Xv[0], in_=xv[0])
            nc.scalar.dma_start(out=Xv[1], in_=xv[1])

            # ---- constant tables (packed; one DMA per ring) ----
            pk1 = consts.tile([128, 644], f32)
            pk2 = consts.tile([128, 2 * RMAX], f16)
            nc.sync.dma_start(out=pk1, in_=pack1_d[:])
            nc.scalar.dma_start(out=pk2, in_=pack2_d[:])
            halfsel = pk1[:, 0:2]
            stbc = pk1[:, 2:258]
            ctbc = pk1[:, 258:514]
            expb = pk1[:, 514:515]
            halfb = pk1[:, 515:516]
            halfones2 = pk1[0:2, 516:644]
            cb = pk2[:, 0:RMAX]
            sbt = pk2[:, RMAX : 2 * RMAX]


            # ---- grayscale accumulate: t = R + r1*G + r2*B; rs = per-b2 row sums ----
            Xc = X[:].rearrange("p (q c) -> p q c", c=3)
            t = work.tile([128, 4, 64], f32)
            rs = work.tile([128, 4], f32)
            nc.vector.scalar_tensor_tensor(
                out=t.rearrange("p a b -> p (a b)"),
                in0=Xc[:, :, 1], scalar=float(R1), in1=Xc[:, :, 0],
                op0=Alu.mult, op1=Alu.add,
            )
            Xb = X[:].rearrange("p (a b c) -> p a b c", a=4, c=3)
            for b2 in range(4):
                nc.vector.scalar_tensor_tensor(
                    out=t[:, b2], in0=Xb[:, b2, :, 2], scalar=float(R2),
                    in1=t[:, b2], op0=Alu.mult, op1=Alu.add,
                    accum_out=rs[:, b2 : b2 + 1],
                )

            # ---- mean: cross-partition sum via PE, broadcast back via PE ----
            csS = psum_m.tile([2, 4, 1], f32)
            nc.tensor.matmul(csS, halfsel, rs, start=True, stop=True)
            Rb = work.tile([2, 4, 64], f32)
            nc.vector.tensor_scalar_mul(
                out=Rb, in0=csS.broadcast_to([2, 4, 64]), scalar1=float(KAVG2)
            )
            avgbc = psum_m.tile([128, 256], f32)
            nc.tensor.matmul(
                avgbc, halfones2, Rb[:].rearrange("p a b -> p (a b)"),
                start=True, stop=True,
            )

            # ---- px = clip(3*gray - 2*avg, 0, 255) ----
            px = work.tile([128, 256], f32)
            nc.vector.scalar_tensor_tensor(
                out=px, in0=t.rearrange("p a b -> p (a b)"), scalar=float(C00),
                in1=avgbc, op0=Alu.mult, op1=Alu.subtract,
            )
            nc.vector.tensor_scalar(
                out=px, in0=px, scalar1=0.0, scalar2=255.0,
                op0=Alu.max, op1=Alu.min,
            )

            # ---- A = (px > 0) * exp(EXP_A*px + EXP_B) ----
            E = work.tile([128, 256], f32)
            nc.scalar.activation(
                out=E, in_=px, func=Act.Exp, bias=expb, scale=float(EXP_A)
            )
            A = work.tile([128, 256], f32)
            nc.vector.scalar_tensor_tensor(
                out=A, in0=px, scalar=0.0, in1=E, op0=Alu.is_gt, op1=Alu.mult
            )

            # ---- P = A*sin(theta), Q = A*cos(theta)  (fp16 for the PE) ----
            P = work.tile([128, 256], f16)
            Q = work.tile([128, 256], f16)
            nc.vector.tensor_mul(out=P, in0=A, in1=stbc)
            nc.vector.tensor_mul(out=Q, in0=A, in1=ctbc)

            # ---- per 2-batch group: Y = P^T@CB + Q^T@SB ; +0.5 ; clip ; DMA ----
            for g in range(4):
                bh, s = divmod(g, 2)
                prt = slice(64 * bh, 64 * (bh + 1))
                col = slice(128 * s, 128 * (s + 1))
                y_ps = psum_y.tile([128, RMAX], f32, tag="y")
                nc.tensor.matmul(y_ps, P[prt, col], cb[prt], start=True, stop=False)
                nc.tensor.matmul(y_ps, Q[prt, col], sbt[prt], start=False, stop=True)

                u = outp.tile([128, RMAX], f32, tag="u")
                nc.scalar.activation(
                    out=u, in_=y_ps, func=Act.Identity, bias=halfb, scale=1.0
                )
                nc.vector.tensor_scalar(
                    out=u, in0=u, scalar1=-32768.0, scalar2=32767.0,
                    op0=Alu.max, op1=Alu.min,
                )

                # main blocks: one DMA per batch-half. HWDGE fans descriptors
                # over the 16 SDMA engines by the DRAM-side outer dim, so keep
                # the 64-wide n dim outermost (a merged [l,n,r] DMA with outer
                # count 2 lands on only 2 engines - measured 5x slower).
                b0 = 4 * bh + 2 * s
                for half, eng in ((0, nc.sync), (1, nc.scalar)):
                    eng.dma_start(
                        out=bass.AP(
                            audio_d, (b0 + half) * NS, [[NUM, 64], [1, NUM]]
                        ),
                        in_=u[64 * half : 64 * (half + 1), 0:NUM],
                    )
                # tails: partition {63, 127} columns [361, 409) -> audio[b, 23104:]
                for half in range(2):
                    nc.gpsimd.dma_start(
                        out=bass.AP(
                            audio_d,
                            (b0 + half) * NS + N * NUM,
                            [[1, 1], [1, RMAX - NUM]],
                        ),
                        in_=u[64 * half + 63 : 64 * half + 64, NUM:RMAX],
                    )

    nc.compile()
    return nc


_NC = None


def _get_nc():
    global _NC
    if _NC is None:
        _NC = build_nc()
    return _NC


LAST_RESULTS = None


def kernel(x: np.ndarray) -> np.ndarray:
    from concourse.bass_utils import run_bass_kernel_spmd

    x = np.ascontiguousarray(np.asarray(x, dtype=np.float32))
    assert x.shape == (B, 64, 64, 3), x.shape

    nc = _get_nc()
    tbl = tables()
    in_maps = []
    for c in range(N_CORES):
        m = {"x": np.ascontiguousarray(x[c * B_LOC : (c + 1) * B_LOC])}
        m.update(tbl)
        in_maps.append(m)

    trace = os.environ.get("BASS_KERNEL_TRACE", "0") == "1"
    res = run_bass_kernel_spmd(
        nc, in_maps, core_ids=list(range(N_CORES)), trace=trace
    )
    global LAST_RESULTS
    LAST_RESULTS = res
    return np.concatenate([r["audio"] for r in res.results], axis=0)


# revision 17
# speedup vs baseline: 1.5526x; 1.1191x over previous
"""Trainium2 Bass kernel: image -> additive-sinusoid audio encoding.

Math (per batch image b):
  gray = 255 * (w . rgb);  rev = flip(gray, rows);  avg = mean(gray)
  px   = clip(3*rev - 2*avg, 0, 255)
  A    = where(px==0, 0, exp(ln10 * (px/160 - 1.5)))            # [M=64 rows, N=64 cols]
  y[t] = sum_m A[m, col(t)] * sin(W[m]*t*dt + PHI0[m]),  col(t) = min(t//361, 63)
  audio= clip(0.5 + 2048*y, -32768, 32767)                       # [ns=23152]

Kernel strategy: t = n*361 + r  =>  angle = theta[i,n] + beta[i,r] (row flip folded
into the host tables), so  sinmat = sin(theta)cos(beta) + cos(theta)sin(beta) and
the gathered einsum becomes dense fp16 matmuls of P/Q = A*sin(theta)/A*cos(theta)
against tiny constant cos/sin(beta) banks. Data-parallel over batch: 8 images per
NeuronCore, layout [128 partitions = (batch-half, image-row), 256 = (b2, col)].
"""

import os

import numpy as np

# ---- problem constants (from the nn.Module definition; input-independent) ----
M = 64
N = 64
FL, FH, FS, T = 80.0, 7600.0, 22050, 1.05
NS = 2 * int(0.5 * FS * T)  # 23152
NUM = NS // N  # 361
RMAX = NS - (N - 1) * NUM  # 409 (last column's sample count)
DT = float(np.float32(1.0 / FS))  # reference rounds dt to f32 (jnp weak typing)
TWO_PI = 2.0 * np.pi
B = 64
N_CORES = 8
B_LOC = B // N_CORES  # 8 images per core
SCALE_SSM = (0.5 / np.sqrt(M)) * 32768.0  # 2048
LN10 = float(np.log(10.0))
EXP_A = LN10 / 160.0
EXP_B = -1.5 * LN10
W0, W1, W2 = 0.2989, 0.5870, 0.1140
C00 = 3.0 * 255.0 * W0  # fold of the 3*255*w0 scale into the gray accumulator
R1 = W1 / W0
R2 = W2 / W0
KAVG2 = 2.0 * 255.0 * W0 / 4096.0  # sum(t) -> 2*avg(gray255) weighting


def _make_tables():
    # LCG phase bank (faithful port, ir starts at 0)
    ia, ic, im = 9301, 49297, 233280
    ir = 0
    phi = []
    for _ in range(M):
        ir = (ir * ia + ic) % im
        phi.append(TWO_PI * ir / im)
    phi32 = np.array(phi, np.float64).astype(np.float32)
    w32 = (TWO_PI * FL * (FH / FL) ** (np.arange(M) / (M - 1))).astype(np.float32)

    # fold the row flip (tf.reverse on axis 1) into the tables: row i uses W[63-i]
    wf = w32[::-1].astype(np.float64)
    phif = phi32[::-1].astype(np.float64)

    n_idx = np.arange(N, dtype=np.float64)
    theta = wf[:, None] * (n_idx[None, :] * NUM * DT) + phif[:, None]  # [64, 64]
    st = np.sin(theta)
    ct = np.cos(theta)

    r_idx = np.arange(RMAX, dtype=np.float64)
    beta = wf[:, None] * (r_idx[None, :] * DT)  # [64, 409]
    cb = np.tile((SCALE_SSM * np.cos(beta)).astype(np.float16), (2, 1))  # [128, RMAX]
    sb = np.tile((SCALE_SSM * np.sin(beta)).astype(np.float16), (2, 1))

    # [p=(bh,i), (b2,n)] broadcast of the theta tables
    stbc = np.tile(st[None, :, None, :], (2, 1, 4, 1)).reshape(128, 256)
    ctbc = np.tile(ct[None, :, None, :], (2, 1, 4, 1)).reshape(128, 256)

    halfsel = np.zeros((128, 2), np.float32)
    halfsel[:64, 0] = 1.0
    halfsel[64:, 1] = 1.0

    # pack1: [halfsel(2) | stbc(256) | ctbc(256) | expb(1) | pad(1) | bcast128(128)]
    # bcast128[p, m] = KAVG2 * (p//64 == m//64): one matmul does the
    # cross-partition mean reduce AND broadcasts it to all 128 partitions.
    pack1 = np.zeros((128, 644), np.float32)
    pack1[:, 0:2] = halfsel
    pack1[:, 2:258] = stbc
    pack1[:, 258:514] = ctbc
    pack1[:, 514] = EXP_B
    blk = np.zeros((128, 128), np.float32)
    blk[:64, :64] = KAVG2
    blk[64:, 64:] = KAVG2
    pack1[:, 516:644] = blk

    # pack2: [cb | sb | ones_row | half_row] fp16 (last two only partition 0)
    extra = np.zeros((128, 128 + RMAX), np.float16)
    extra[0, 0:128] = 1.0
    extra[0, 128 : 128 + RMAX] = 0.5
    pack2 = np.concatenate([cb, sb, extra], axis=1)  # [128, 2*RMAX + 537]

    return {"pack1": pack1, "pack2": pack2}


_TABLES = None


def tables():
    global _TABLES
    if _TABLES is None:
        _TABLES = _make_tables()
    return _TABLES


def build_nc():
    import concourse.bacc as bacc
    import concourse.bass as bass
    import concourse.mybir as mybir
    import concourse.tile as tile

    f32 = mybir.dt.float32
    f16 = mybir.dt.float16
    Alu = mybir.AluOpType
    Act = mybir.ActivationFunctionType

    nc = bacc.Bacc(
        "TRN2",
        target_bir_lowering=False,
        debug=False,
        num_devices=N_CORES,
        enable_asserts=False,
    )

    x_d = nc.dram_tensor("x", [B_LOC, 64, 64, 3], f32, kind="ExternalInput")
    pack1_d = nc.dram_tensor("pack1", [128, 644], f32, kind="ExternalInput")
    pack2_d = nc.dram_tensor(
        "pack2", [128, 2 * RMAX + 128 + RMAX], f16, kind="ExternalInput"
    )
    audio_d = nc.dram_tensor("audio", [B_LOC, NS], f32, kind="ExternalOutput")

    with tile.TileContext(nc) as tc:
        with (
            tc.tile_pool(name="consts", bufs=1) as consts,
            tc.tile_pool(name="work", bufs=1) as work,
            tc.tile_pool(name="outp", bufs=4) as outp,
            tc.tile_pool(name="psum_y", bufs=4, space=bass.MemorySpace.PSUM) as psum_y,
            tc.tile_pool(name="psum_m", bufs=1, space=bass.MemorySpace.PSUM) as psum_m,
        ):
            # ---- input image: [p=(bh,i), (b2, j, c)]; one DMA per batch half,
            # split across the two physical HWDGE rings (sync / scalar) ----
            X = work.tile([128, 768], f32)
            xv = x_d[:].rearrange("(bh b2) i j c -> bh i b2 j c", bh=2)
            Xv = X[:].rearrange("(bh i) (b2 j c) -> bh i b2 j c", bh=2, b2=4, c=3)
            nc.sync.dma_start(out=Xv[0], in_=xv[0])
            nc.scalar.dma_start(out=Xv[1], in_=xv[1])

            # ---- constant tables (packed; one DMA per ring) ----
            pk1 = consts.tile([128, 644], f32)
            pk2 = consts.tile([128, 2 * RMAX + 128 + RMAX], f16)
            nc.sync.dma_start(out=pk1, in_=pack1_d[:])
            nc.scalar.dma_start(out=pk2, in_=pack2_d[:])
            halfsel = pk1[:, 0:2]
            stbc = pk1[:, 2:258]
            ctbc = pk1[:, 258:514]
            expb = pk1[:, 514:515]
            bcast128 = pk1[:, 516:644]
            cb = pk2[:, 0:RMAX]
            sbt = pk2[:, RMAX : 2 * RMAX]
            ones_row = pk2[0:1, 2 * RMAX : 2 * RMAX + 128]
            half_row = pk2[0:1, 2 * RMAX + 128 : 2 * RMAX + 128 + RMAX]

            # ---- PSUM pre-fill: y = 0.5 via K=1 matmul per group; runs during
            # the input/elementwise phase (only depends on pack2) ----
            y_tiles = []
            for g in range(4):
                y_ps = psum_y.tile([128, NUM], f32, tag="y")
                nc.tensor.matmul(y_ps, ones_row, half_row[0:1, 0:NUM], start=True, stop=False)
                y_tiles.append(y_ps)

            # ---- grayscale accumulate: t = R + r1*G + r2*B; rs = per-b2 sums ----
            Xc = X[:].rearrange("p (q c) -> p q c", c=3)
            t = work.tile([128, 4, 64], f32)
            rs = work.tile([128, 4], f32)
            nc.vector.scalar_tensor_tensor(
                out=t.rearrange("p a b -> p (a b)"),
                in0=Xc[:, :, 1], scalar=float(R1), in1=Xc[:, :, 0],
                op0=Alu.mult, op1=Alu.add,
            )
            Xb = X[:].rearrange("p (a b c) -> p a b c", a=4, c=3)
            for b2 in range(4):
                nc.vector.scalar_tensor_tensor(
                    out=t[:, b2], in0=Xb[:, b2, :, 2], scalar=float(R2),
                    in1=t[:, b2], op0=Alu.mult, op1=Alu.add,
                    accum_out=rs[:, b2 : b2 + 1],
                )

            # ---- mean: one matmul reduces across partitions AND broadcasts:
            # csS2[p, b2] = KAVG2 * sum_{p' in half(p)} rs[p', b2] = 2*avg ----
            csS2 = psum_m.tile([128, 4], f32)
            nc.tensor.matmul(csS2, bcast128, rs, start=True, stop=True)

            # ---- per column-half s (b2 pair): px -> A -> P/Q -> matmul -> out
            # (pipelines the s=1 elementwise under s=0's PE/DMA work) ----
            px = work.tile([128, 4, 64], f32)
            E = work.tile([128, 4, 64], f32)
            A = work.tile([128, 4, 64], f32)
            P = work.tile([128, 256], f16)
            Q = work.tile([128, 256], f16)
            tailps = psum_m.tile([2, 4, RMAX - NUM], f32)
            Pv = P[:].rearrange("p (a b) -> p a b", b=64)
            Qv = Q[:].rearrange("p (a b) -> p a b", b=64)
            for s in range(2):
                b2s = slice(2 * s, 2 * s + 2)
                nc.vector.scalar_tensor_tensor(
                    out=px[:, b2s], in0=t[:, b2s], scalar=float(C00),
                    in1=csS2[:, b2s].broadcast_to([128, 2, 64]),
                    op0=Alu.mult, op1=Alu.subtract,
                )
                nc.vector.tensor_scalar(
                    out=px[:, b2s], in0=px[:, b2s], scalar1=0.0, scalar2=255.0,
                    op0=Alu.max, op1=Alu.min,
                )
                nc.scalar.activation(
                    out=E[:, b2s], in_=px[:, b2s], func=Act.Exp,
                    bias=expb, scale=float(EXP_A),
                )
                nc.vector.scalar_tensor_tensor(
                    out=A[:, b2s], in0=px[:, b2s], scalar=0.0, in1=E[:, b2s],
                    op0=Alu.is_gt, op1=Alu.mult,
                )
                nc.vector.tensor_mul(out=Pv[:, b2s], in0=A[:, b2s], in1=stbc.rearrange("p (a b) -> p a b", b=64)[:, b2s])
                nc.gpsimd.tensor_mul(out=Qv[:, b2s], in0=A[:, b2s], in1=ctbc.rearrange("p (a b) -> p a b", b=64)[:, b2s])

                col = slice(128 * s, 128 * (s + 1))
                for bh in range(2):
                    g = 2 * bh + s
                    prt = slice(64 * bh, 64 * (bh + 1))
                    y_ps = y_tiles[g]
                    nc.tensor.matmul(
                        y_ps, P[prt, col], cb[prt, 0:NUM],
                        start=False, stop=False,
                    )
                    nc.tensor.matmul(
                        y_ps, Q[prt, col], sbt[prt, 0:NUM],
                        start=False, stop=True,
                    )

                    # tail samples (n=63, r>=361): tiny matmuls on the
                    # n=63 columns of P/Q into a dedicated [2, g, 48] psum
                    ctail = slice(128 * s + 63, 128 * s + 128, 64)
                    nc.tensor.matmul(
                        tailps[:, g], ones_row[0:1, 0:2],
                        half_row[0:1, 0 : RMAX - NUM],
                        start=True, stop=False,
                    )
                    nc.tensor.matmul(
                        tailps[:, g], P[prt, ctail], cb[prt, NUM:RMAX],
                        start=False, stop=False,
                    )
                    nc.tensor.matmul(
                        tailps[:, g], Q[prt, ctail], sbt[prt, NUM:RMAX],
                        start=False, stop=True,
                    )

                    u = outp.tile([128, NUM], f32, tag="u")
                    nc.vector.tensor_scalar(
                        out=u, in0=y_ps[:, 0:NUM],
                        scalar1=-32768.0, scalar2=32767.0,
                        op0=Alu.max, op1=Alu.min,
                    )
                    b0 = 4 * bh + 2 * s
                    # main blocks: one DMA per batch-half; keep the 64-wide n
                    # dim outermost (HWDGE fans descriptors over the 16 SDMA
                    # engines by the DRAM-side outer dim)
                    for half, eng in ((0, nc.sync), (1, nc.scalar)):
                        eng.dma_start(
                            out=bass.AP(
                                audio_d, (b0 + half) * NS, [[NUM, 64], [1, NUM]]
                            ),
                            in_=u[64 * half : 64 * (half + 1), :],
                        )
            # clip + store all 8 tails (batch b = 2g+l) in one op + one DMA
            TTs = outp.tile([2, 4, RMAX - NUM], f32)
            nc.vector.tensor_scalar(
                out=TTs, in0=tailps, scalar1=-32768.0, scalar2=32767.0,
                op0=Alu.max, op1=Alu.min,
            )
            nc.gpsimd.dma_start(
                out=bass.AP(
                    audio_d, N * NUM, [[NS, 2], [2 * NS, 4], [1, RMAX - NUM]]
                ),
                in_=TTs,
            )

    nc.compile()
    return nc


_NC = None


def _get_nc():
    global _NC
    if _NC is None:
        _NC = build_nc()
    return _NC


LAST_RESULTS = None


def kernel(x: np.ndarray) -> np.ndarray:
    from concourse.bass_utils import run_bass_kernel_spmd

    x = np.ascontiguousarray(np.asarray(x, dtype=np.float32))
    assert x.shape == (B, 64, 64, 3), x.shape

    nc = _get_nc()
    tbl = tables()
    in_maps = []
    for c in range(N_CORES):
        m = {"x": np.ascontiguousarray(x[c * B_LOC : (c + 1) * B_LOC])}
        m.update(tbl)
        in_maps.append(m)

    trace = os.environ.get("BASS_KERNEL_TRACE", "0") == "1"
    res = run_bass_kernel_spmd(
        nc, in_maps, core_ids=list(range(N_CORES)), trace=trace
    )
    global LAST_RESULTS
    LAST_RESULTS = res
    return np.concatenate([r["audio"] for r in res.results], axis=0)


# revision 20
# speedup vs baseline: 1.5704x; 1.0115x over previous
"""Trainium2 Bass kernel: image -> additive-sinusoid audio encoding.

Math (per batch image b):
  gray = 255 * (w . rgb);  rev = flip(gray, rows);  avg = mean(gray)
  px   = clip(3*rev - 2*avg, 0, 255)
  A    = where(px==0, 0, exp(ln10 * (px/160 - 1.5)))            # [M=64 rows, N=64 cols]
  y[t] = sum_m A[m, col(t)] * sin(W[m]*t*dt + PHI0[m]),  col(t) = min(t//361, 63)
  audio= clip(0.5 + 2048*y, -32768, 32767)                       # [ns=23152]

Kernel strategy: t = n*361 + r  =>  angle = theta[i,n] + beta[i,r] (row flip folded
into the host tables), so  sinmat = sin(theta)cos(beta) + cos(theta)sin(beta) and
the gathered einsum becomes dense fp16 matmuls of P/Q = A*sin(theta)/A*cos(theta)
against tiny constant cos/sin(beta) banks. Data-parallel over batch: 8 images per
NeuronCore, layout [128 partitions = (batch-half, image-row), 256 = (b2, col)].
"""

import os

import numpy as np

# ---- problem constants (from the nn.Module definition; input-independent) ----
M = 64
N = 64
FL, FH, FS, T = 80.0, 7600.0, 22050, 1.05
NS = 2 * int(0.5 * FS * T)  # 23152
NUM = NS // N  # 361
RMAX = NS - (N - 1) * NUM  # 409 (last column's sample count)
DT = float(np.float32(1.0 / FS))  # reference rounds dt to f32 (jnp weak typing)
TWO_PI = 2.0 * np.pi
B = 64
N_CORES = 8
B_LOC = B // N_CORES  # 8 images per core
SCALE_SSM = (0.5 / np.sqrt(M)) * 32768.0  # 2048
LN10 = float(np.log(10.0))
EXP_A = LN10 / 160.0
EXP_B = -1.5 * LN10
W0, W1, W2 = 0.2989, 0.5870, 0.1140
C00 = 3.0 * 255.0 * W0  # fold of the 3*255*w0 scale into the gray accumulator
R1 = W1 / W0
R2 = W2 / W0
KAVG2 = 2.0 * 255.0 * W0 / 4096.0  # sum(t) -> 2*avg(gray255) weighting


def _make_tables():
    # LCG phase bank (faithful port, ir starts at 0)
    ia, ic, im = 9301, 49297, 233280
    ir = 0
    phi = []
    for _ in range(M):
        ir = (ir * ia + ic) % im
        phi.append(TWO_PI * ir / im)
    phi32 = np.array(phi, np.float64).astype(np.float32)
    w32 = (TWO_PI * FL * (FH / FL) ** (np.arange(M) / (M - 1))).astype(np.float32)

    # fold the row flip (tf.reverse on axis 1) into the tables: row i uses W[63-i]
    wf = w32[::-1].astype(np.float64)
    phif = phi32[::-1].astype(np.float64)

    n_idx = np.arange(N, dtype=np.float64)
    theta = wf[:, None] * (n_idx[None, :] * NUM * DT) + phif[:, None]  # [64, 64]
    st = np.sin(theta)
    ct = np.cos(theta)

    r_idx = np.arange(RMAX, dtype=np.float64)
    beta = wf[:, None] * (r_idx[None, :] * DT)  # [64, 409]
    cb = np.tile((SCALE_SSM * np.cos(beta)).astype(np.float16), (2, 1))  # [128, RMAX]
    sb = np.tile((SCALE_SSM * np.sin(beta)).astype(np.float16), (2, 1))

    # [p=(bh,i), (b2,n)] broadcast of the theta tables
    stbc = np.tile(st[None, :, None, :], (2, 1, 4, 1)).reshape(128, 256)
    ctbc = np.tile(ct[None, :, None, :], (2, 1, 4, 1)).reshape(128, 256)

    # pack1: [stbc(256) | ctbc(256) | expb(1)] fp32
    pack1 = np.zeros((128, 513), np.float32)
    pack1[:, 0:256] = stbc
    pack1[:, 256:512] = ctbc
    pack1[:, 512] = EXP_B

    # pack2 (fp16): [cb | sb | ones_row+half_row(537, partition 0) | bcast128(128)]
    # bcast128[p, m] = KAVG2 * (p//64 == m//64): one matmul does the
    # cross-partition mean reduce AND broadcasts it to all 128 partitions.
    extra = np.zeros((128, 128 + RMAX), np.float16)
    extra[0, 0:128] = 1.0
    extra[0, 128 : 128 + RMAX] = 0.5
    blk = np.zeros((128, 128), np.float16)
    blk[:64, :64] = 1.0
    blk[64:, 64:] = 1.0
    pack2 = np.concatenate([cb, sb, extra, blk.astype(np.float16)], axis=1)

    return {"pack1": pack1, "pack2": pack2}


_TABLES = None


def tables():
    global _TABLES
    if _TABLES is None:
        _TABLES = _make_tables()
    return _TABLES


def build_nc():
    import concourse.bacc as bacc
    import concourse.bass as bass
    import concourse.mybir as mybir
    import concourse.tile as tile

    f32 = mybir.dt.float32
    f16 = mybir.dt.float16
    Alu = mybir.AluOpType
    Act = mybir.ActivationFunctionType

    nc = bacc.Bacc(
        "TRN2",
        target_bir_lowering=False,
        debug=False,
        num_devices=N_CORES,
        enable_asserts=False,
    )

    x_d = nc.dram_tensor("x", [B_LOC, 64, 64, 3], f32, kind="ExternalInput")
    pack1_d = nc.dram_tensor("pack1", [128, 513], f32, kind="ExternalInput")
    pack2_d = nc.dram_tensor(
        "pack2", [128, 3 * RMAX + 256], f16, kind="ExternalInput"
    )
    audio_d = nc.dram_tensor("audio", [B_LOC, NS], f32, kind="ExternalOutput")

    with tile.TileContext(nc) as tc:
        with (
            tc.tile_pool(name="consts", bufs=1) as consts,
            tc.tile_pool(name="work", bufs=1) as work,
            tc.tile_pool(name="outp", bufs=8) as outp,
            tc.tile_pool(name="psum_y", bufs=4, space=bass.MemorySpace.PSUM) as psum_y,
            tc.tile_pool(name="psum_m", bufs=1, space=bass.MemorySpace.PSUM) as psum_m,
        ):
            # ---- input image: [p=(bh,i), (b2, j, c)]; one DMA per batch half,
            # split across the two physical HWDGE rings (sync / scalar) ----
            X = work.tile([128, 768], f32)
            xv = x_d[:].rearrange("(bh b2) i j c -> bh i b2 j c", bh=2)
            Xv = X[:].rearrange("(bh i) (b2 j c) -> bh i b2 j c", bh=2, b2=4, c=3)
            for bh, eng in ((0, nc.sync), (1, nc.scalar)):
                for q in range(2):
                    eng.dma_start(
                        out=Xv[bh][:, 2 * q : 2 * q + 2],
                        in_=xv[bh][:, 2 * q : 2 * q + 2],
                    )

            # ---- constant tables (pack2 on the scalar ring, pack1 on SWDGE) ----
            pk1 = consts.tile([128, 513], f32)
            pk2 = consts.tile([128, 3 * RMAX + 256], f16)
            nc.scalar.dma_start(out=pk2, in_=pack2_d[:])
            nc.gpsimd.dma_start(out=pk1, in_=pack1_d[:])
            stbc = pk1[:, 0:256]
            ctbc = pk1[:, 256:512]
            expb = pk1[:, 512:513]
            cb = pk2[:, 0:RMAX]
            sbt = pk2[:, RMAX : 2 * RMAX]
            ones_row = pk2[0:1, 2 * RMAX : 2 * RMAX + 128]
            half_row = pk2[0:1, 2 * RMAX + 128 : 3 * RMAX + 128]
            bcast128 = pk2[:, 3 * RMAX + 128 : 3 * RMAX + 256]

            # ---- PSUM pre-fill: y = 0.5 via K=1 matmul per group; runs during
            # the input/elementwise phase (only depends on pack2) ----
            y_tiles = []
            for g in range(4):
                y_ps = psum_y.tile([128, NUM], f32, tag="y")
                nc.tensor.matmul(y_ps, ones_row, half_row[0:1, 0:NUM], start=True, stop=False)
                y_tiles.append(y_ps)

            # ---- grayscale accumulate: t = R + r1*G + r2*B; rs = per-b2 sums ----
            Xc = X[:].rearrange("p (q c) -> p q c", c=3)
            t = work.tile([128, 4, 64], f32)
            rs = work.tile([128, 4], f32)
            rs16 = work.tile([128, 4], f16)
            nc.vector.scalar_tensor_tensor(
                out=t.rearrange("p a b -> p (a b)"),
                in0=Xc[:, :, 1], scalar=float(R1), in1=Xc[:, :, 0],
                op0=Alu.mult, op1=Alu.add,
            )
            Xb = X[:].rearrange("p (a b c) -> p a b c", a=4, c=3)
            for b2 in range(4):
                nc.vector.scalar_tensor_tensor(
                    out=t[:, b2], in0=Xb[:, b2, :, 2], scalar=float(R2),
                    in1=t[:, b2], op0=Alu.mult, op1=Alu.add,
                    accum_out=rs[:, b2 : b2 + 1],
                )
            nc.vector.tensor_scalar_mul(out=rs16, in0=rs, scalar1=float(KAVG2))

            # ---- mean: one matmul reduces across partitions AND broadcasts:
            # csS2[p, b2] = KAVG2 * sum_{p' in half(p)} rs[p', b2] = 2*avg ----
            csS2 = psum_m.tile([128, 4], f32)
            nc.tensor.matmul(csS2, bcast128, rs16, start=True, stop=True)

            # ---- per column-half s (b2 pair): px -> A -> P/Q -> matmul -> out
            # (pipelines the s=1 elementwise under s=0's PE/DMA work) ----
            px = work.tile([128, 4, 64], f32)
            E = work.tile([128, 4, 64], f32)
            A = work.tile([128, 4, 64], f32)
            P = work.tile([128, 256], f16)
            Q = work.tile([128, 256], f16)
            tailps = psum_m.tile([2, 4, RMAX - NUM], f32)
            Pv = P[:].rearrange("p (a b) -> p a b", b=64)
            Qv = Q[:].rearrange("p (a b) -> p a b", b=64)
            for s in range(2):
                b2s = slice(2 * s, 2 * s + 2)
                nc.vector.scalar_tensor_tensor(
                    out=px[:, b2s], in0=t[:, b2s], scalar=float(C00),
                    in1=csS2[:, b2s].broadcast_to([128, 2, 64]),
                    op0=Alu.mult, op1=Alu.subtract,
                )
                nc.vector.tensor_scalar(
                    out=px[:, b2s], in0=px[:, b2s], scalar1=0.0, scalar2=255.0,
                    op0=Alu.max, op1=Alu.min,
                )
                nc.scalar.activation(
                    out=E[:, b2s], in_=px[:, b2s], func=Act.Exp,
                    bias=expb, scale=float(EXP_A),
                )
                nc.vector.scalar_tensor_tensor(
                    out=A[:, b2s], in0=px[:, b2s], scalar=0.0, in1=E[:, b2s],
                    op0=Alu.is_gt, op1=Alu.mult,
                )
                nc.vector.tensor_mul(out=Pv[:, b2s], in0=A[:, b2s], in1=stbc.rearrange("p (a b) -> p a b", b=64)[:, b2s])
                nc.gpsimd.tensor_mul(out=Qv[:, b2s], in0=A[:, b2s], in1=ctbc.rearrange("p (a b) -> p a b", b=64)[:, b2s])

                col = slice(128 * s, 128 * (s + 1))
                for bh in range(2):
                    g = 2 * bh + s
                    prt = slice(64 * bh, 64 * (bh + 1))
                    y_ps = y_tiles[g]
                    nc.tensor.matmul(
                        y_ps, P[prt, col], cb[prt, 0:NUM],
                        start=False, stop=False,
                    )
                    nc.tensor.matmul(
                        y_ps, Q[prt, col], sbt[prt, 0:NUM],
                        start=False, stop=True,
                    )

                    # tail samples (n=63, r>=361): tiny matmuls on the
                    # n=63 columns of P/Q into a dedicated [2, g, 48] psum
                    ctail = slice(128 * s + 63, 128 * s + 128, 64)
                    nc.tensor.matmul(
                        tailps[:, g], ones_row[0:1, 0:2],
                        half_row[0:1, 0 : RMAX - NUM],
                        start=True, stop=False,
                    )
                    nc.tensor.matmul(
                        tailps[:, g], P[prt, ctail], cb[prt, NUM:RMAX],
                        start=False, stop=False,
                    )
                    nc.tensor.matmul(
                        tailps[:, g], Q[prt, ctail], sbt[prt, NUM:RMAX],
                        start=False, stop=True,
                    )

                    u = outp.tile([128, NUM], f32, tag="u")
                    nc.vector.tensor_scalar(
                        out=u, in0=y_ps[:, 0:NUM],
                        scalar1=-32768.0, scalar2=32767.0,
                        op0=Alu.max, op1=Alu.min,
                    )
                    b0 = 4 * bh + 2 * s
                    # main blocks: one DMA per batch-half; keep the 64-wide n
                    # dim outermost (HWDGE fans descriptors over the 16 SDMA
                    # engines by the DRAM-side outer dim)
                    for half, eng in ((0, nc.sync), (1, nc.scalar)):
                        eng.dma_start(
                            out=bass.AP(
                                audio_d, (b0 + half) * NS, [[NUM, 64], [1, NUM]]
                            ),
                            in_=u[64 * half : 64 * (half + 1), :],
                        )
            # clip + store all 8 tails (batch b = 2g+l) in one op + one DMA
            TTs = outp.tile([2, 4, RMAX - NUM], f32)
            nc.vector.tensor_scalar(
                out=TTs, in0=tailps, scalar1=-32768.0, scalar2=32767.0,
                op0=Alu.max, op1=Alu.min,
            )
            nc.sync.dma_start(
                out=bass.AP(
                    audio_d, N * NUM, [[NS, 2], [2 * NS, 4], [1, RMAX - NUM]]
                ),
                in_=TTs,
            )

    nc.compile()
    return nc


_NC = None


def _get_nc():
    global _NC
    if _NC is None:
        _NC = build_nc()
    return _NC


LAST_RESULTS = None


def kernel(x: np.ndarray) -> np.ndarray:
    from concourse.bass_utils import run_bass_kernel_spmd

    x = np.ascontiguousarray(np.asarray(x, dtype=np.float32))
    assert x.shape == (B, 64, 64, 3), x.shape

    nc = _get_nc()
    tbl = tables()
    in_maps = []
    for c in range(N_CORES):
        m = {"x": np.ascontiguousarray(x[c * B_LOC : (c + 1) * B_LOC])}
        m.update(tbl)
        in_maps.append(m)

    trace = os.environ.get("BASS_KERNEL_TRACE", "0") == "1"
    res = run_bass_kernel_spmd(
        nc, in_maps, core_ids=list(range(N_CORES)), trace=trace
    )
    global LAST_RESULTS
    LAST_RESULTS = res
    return np.concatenate([r["audio"] for r in res.results], axis=0)
